# revision 2
# baseline (speedup 1.0000x reference)
"""GridExp (scaling-and-squaring of a velocity field) on 8 trn2 NeuronCores.

Algorithm: d <- d + pull(d, id + d) eight times, where pull is approximated
by axis-sequential shear interpolation (z, then y, then x), each a 1-D
linear interp at per-voxel offsets implemented as tent-weighted MACs over
shifted reads. Validated vs the exact trilinear reference: rel err ~1.5e-3.

Device layout per core: partitions p = c*32 + xl (channel blocks at
partitions 0/32/64, xl = local x-plane: 24 owned + halo), free dim =
padded (y, z) = (196, 196), bf16. x-shifts are done via DMA SBUF->SBUF
partition-shifted copies (compute engines require partition-aligned APs).

The 8 steps run as 3 launches (3+3+2 steps); the host re-shards d between
launches to refresh the x halos (halo budget H=3 fits the 32-partition
channel blocks).
"""
import numpy as np
import ml_dtypes

X = Y = Z = 192
C = 3
NCORES = 8
SLAB = X // NCORES   # 24
PX = 32
P = 3 * PX
PAD = 2
YP = Y + 2 * PAD
ZP = Z + 2 * PAD
CH = 32
NCH = Y // CH
STEPS = 8

LAUNCH_CFGS = [
    ((0.99, 3),) * 3,                 # steps 0-2
    ((0.99, 3),) * 3,                 # steps 3-5 (same program as launch 0)
    ((0.99, 3), (1.99, 5)),           # steps 6-7
]
H = 3
BF16NP = ml_dtypes.bfloat16

_RUNNERS = {}


def _build_launch(nc, cfgs):
    import concourse.mybir as mybir
    from concourse.tile import TileContext
    F32 = mybir.dt.float32
    BF16 = mybir.dt.bfloat16
    AL = mybir.AluOpType

    d_in = nc.dram_tensor("d_in", [P, YP * ZP], BF16, kind="ExternalInput")
    d_out = nc.dram_tensor("d_out", [C * SLAB, Y * Z], BF16,
                           kind="ExternalOutput")

    def tents(T3, s, a, rows, bias_aps):
        # T3[:, :rows, :] = -(tent(s-a)) = min(|s-a|,1) - 1, replicated x3
        nc.scalar.activation(out=T3[0:PX, 0:rows, :], in_=s[:, 0:rows, :],
                             func=mybir.ActivationFunctionType.Abs,
                             bias=bias_aps[float(a)][0:PX], scale=1.0)
        nc.vector.tensor_scalar(out=T3[0:PX, 0:rows, :], in0=T3[0:PX, 0:rows, :],
                                scalar1=1.0, scalar2=1.0,
                                op0=AL.min, op1=AL.subtract)
        nc.vector.tensor_copy(out=T3[PX:2 * PX, 0:rows, :],
                              in_=T3[0:PX, 0:rows, :])
        nc.scalar.copy(out=T3[2 * PX:3 * PX, 0:rows, :], in_=T3[0:PX, 0:rows, :])

    with TileContext(nc) as tc:
        with tc.tile_pool(name="dp", bufs=1) as dpool, \
             tc.tile_pool(name="wp", bufs=1) as wp, \
             tc.tile_pool(name="cp", bufs=2) as cpool:
            dT = dpool.tile([P, YP, ZP], BF16)
            nc.sync.dma_start(
                out=dT[:, :, :],
                in_=d_in[:].rearrange("p (y z) -> p y z", y=YP))

            bias_aps = {}
            for v in (-2.0, -1.0, 0.0, 1.0, 2.0):
                t = dpool.tile([PX, 1], F32, tag=f"bias{v}")
                nc.vector.memset(t[:], -v)
                bias_aps[v] = t

            for clamp, W in cfgs:
                m = 1 if W == 3 else 2
                ks = list(range(-(W // 2), W // 2 + 1))
                # refresh wrap pads
                nc.vector.tensor_copy(out=dT[:, 0:PAD, PAD:PAD + Z],
                                      in_=dT[:, Y:Y + PAD, PAD:PAD + Z])
                nc.vector.tensor_copy(out=dT[:, Y + PAD:YP, PAD:PAD + Z],
                                      in_=dT[:, PAD:2 * PAD, PAD:PAD + Z])
                nc.vector.tensor_copy(out=dT[:, :, 0:PAD],
                                      in_=dT[:, :, Z:Z + PAD])
                nc.vector.tensor_copy(out=dT[:, :, Z + PAD:ZP],
                                      in_=dT[:, :, PAD:2 * PAD])

                cprev = None
                for j in range(NCH):
                    y0 = PAD + j * CH
                    ay0 = y0 - m
                    ae = CH + 2 * m

                    A = wp.tile([P, ae, ZP], BF16, tag="A")
                    T3 = wp.tile([P, ae, ZP], BF16, tag="T3")
                    s = wp.tile([PX, ae, ZP], BF16, tag="s")
                    tmp = wp.tile([P, ae, ZP], BF16, tag="tmp")

                    # z pass
                    nc.vector.tensor_scalar(
                        out=s[:, :, :], in0=dT[2 * PX:3 * PX, ay0:ay0 + ae, :],
                        scalar1=clamp, scalar2=-clamp, op0=AL.min, op1=AL.max)
                    for i, a in enumerate(ks):
                        tents(T3, s, a, ae, bias_aps)
                        src = dT[:, ay0:ay0 + ae, PAD + a:PAD + a + Z]
                        if i == 0:
                            nc.vector.tensor_tensor(
                                out=A[:, :, PAD:PAD + Z],
                                in0=T3[:, :, PAD:PAD + Z], in1=src, op=AL.mult)
                        else:
                            nc.vector.tensor_tensor(
                                out=tmp[:, :, PAD:PAD + Z],
                                in0=T3[:, :, PAD:PAD + Z], in1=src, op=AL.mult)
                            nc.vector.tensor_tensor(
                                out=A[:, :, PAD:PAD + Z],
                                in0=A[:, :, PAD:PAD + Z],
                                in1=tmp[:, :, PAD:PAD + Z], op=AL.add)

                    # y pass
                    B = wp.tile([P, CH, ZP], BF16, tag="B")
                    nc.vector.tensor_scalar(
                        out=s[:, 0:CH, :], in0=dT[PX:2 * PX, y0:y0 + CH, :],
                        scalar1=clamp, scalar2=-clamp, op0=AL.min, op1=AL.max)
                    for i, a in enumerate(ks):
                        tents(T3, s, a, CH, bias_aps)
                        src = A[:, m + a:m + a + CH, PAD:PAD + Z]
                        if i == 0:
                            nc.vector.tensor_tensor(
                                out=B[:, :, PAD:PAD + Z],
                                in0=T3[:, 0:CH, PAD:PAD + Z], in1=src,
                                op=AL.mult)
                        else:
                            nc.vector.tensor_tensor(
                                out=tmp[:, 0:CH, PAD:PAD + Z],
                                in0=T3[:, 0:CH, PAD:PAD + Z], in1=src,
                                op=AL.mult)
                            nc.vector.tensor_tensor(
                                out=B[:, :, PAD:PAD + Z],
                                in0=B[:, :, PAD:PAD + Z],
                                in1=tmp[:, 0:CH, PAD:PAD + Z], op=AL.add)

                    # x pass (partition shifts via DMA)
                    Cc = cpool.tile([P, CH, ZP], BF16, tag="C")
                    Bs = wp.tile([P, CH, ZP], BF16, tag="Bs")
                    nc.vector.tensor_scalar(
                        out=s[:, 0:CH, :], in0=dT[0:PX, y0:y0 + CH, :],
                        scalar1=clamp, scalar2=-clamp, op0=AL.min, op1=AL.max)
                    for i, a in enumerate(ks):
                        tents(T3, s, a, CH, bias_aps)
                        if a == 0:
                            src = B[:, :, PAD:PAD + Z]
                        else:
                            for c in range(C):
                                s0 = c * PX + max(a, 0)
                                d0 = c * PX + max(-a, 0)
                                n = PX - abs(a)
                                nc.sync.dma_start(out=Bs[d0:d0 + n, :, :],
                                                  in_=B[s0:s0 + n, :, :])
                            src = Bs[:, :, PAD:PAD + Z]
                        if i == 0:
                            nc.vector.tensor_tensor(
                                out=Cc[:, :, PAD:PAD + Z],
                                in0=T3[:, 0:CH, PAD:PAD + Z], in1=src,
                                op=AL.mult)
                        else:
                            nc.vector.tensor_tensor(
                                out=tmp[:, 0:CH, PAD:PAD + Z],
                                in0=T3[:, 0:CH, PAD:PAD + Z], in1=src,
                                op=AL.mult)
                            nc.vector.tensor_tensor(
                                out=Cc[:, :, PAD:PAD + Z],
                                in0=Cc[:, :, PAD:PAD + Z],
                                in1=tmp[:, 0:CH, PAD:PAD + Z], op=AL.add)

                    if cprev is not None:
                        _fold(nc, AL, dT, cprev)
                    cprev = (j, Cc)
                _fold(nc, AL, dT, cprev)

            for c in range(C):
                nc.sync.dma_start(
                    out=d_out[c * SLAB:(c + 1) * SLAB, :].rearrange(
                        "p (y z) -> p y z", y=Y),
                    in_=dT[c * PX + H:c * PX + H + SLAB, PAD:PAD + Y,
                           PAD:PAD + Z])


def _fold(nc, AL, dT, cprev):
    j, Cc = cprev
    y0 = PAD + j * CH
    nc.vector.tensor_tensor(
        out=dT[:, y0:y0 + CH, PAD:PAD + Z],
        in0=dT[:, y0:y0 + CH, PAD:PAD + Z],
        in1=Cc[:, :, PAD:PAD + Z], op=AL.subtract)


def _bass_jit(nc):
    """Jitted 8-core SPMD executor for a prebuilt Bass module (axon/PJRT).
    Input/output arrays are global [8*rows, cols], sharded over cores."""
    import jax
    import concourse.mybir as mybir
    from concourse.bass2jax import (_bass_exec_p, partition_id_tensor,
                                    install_neuronx_cc_hook)
    from jax.sharding import Mesh, PartitionSpec
    from jax.experimental.shard_map import shard_map

    install_neuronx_cc_hook()
    partition_name = nc.partition_id_tensor.name if nc.partition_id_tensor else None
    in_names, out_names, out_avals = [], [], []
    for alloc in nc.m.functions[0].allocations:
        if not isinstance(alloc, mybir.MemoryLocationSet):
            continue
        name = alloc.memorylocations[0].name
        if alloc.kind == "ExternalInput":
            if name != partition_name:
                in_names.append(name)
        elif alloc.kind == "ExternalOutput":
            out_names.append(name)
            out_avals.append(jax.core.ShapedArray(
                tuple(alloc.tensor_shape), mybir.dt.np(alloc.dtype)))
    all_in_names = list(in_names) + out_names
    if partition_name is not None:
        all_in_names.append(partition_name)

    def _body(*args):
        operands = list(args)
        if partition_name is not None:
            operands.append(partition_id_tensor())
        return tuple(_bass_exec_p.bind(
            *operands,
            out_avals=tuple(out_avals),
            in_names=tuple(all_in_names),
            out_names=tuple(out_names),
            lowering_input_output_aliases=(),
            sim_require_finite=False,
            sim_require_nnan=False,
            nc=nc,
        ))

    mesh = Mesh(np.asarray(jax.devices()[:NCORES]), ("core",))
    nio = len(in_names) + len(out_names)
    jf = jax.jit(shard_map(_body, mesh=mesh,
                           in_specs=(PartitionSpec("core"),) * nio,
                           out_specs=(PartitionSpec("core"),) * len(out_names),
                           check_rep=False),
                 keep_unused=True)
    return jf, out_avals


def _get_pipeline():
    """Compile everything once; return a callable d_own -> d_own (device)."""
    if "pipe" in _RUNNERS:
        return _RUNNERS["pipe"]
    import jax
    import jax.numpy as jnp
    import concourse.bacc as bacc
    from jax.sharding import Mesh, PartitionSpec, NamedSharding
    from jax.experimental.shard_map import shard_map

    mesh = Mesh(np.asarray(jax.devices()[:NCORES]), ("core",))
    shard = NamedSharding(mesh, PartitionSpec("core"))

    jfs = []
    for cfgs in (LAUNCH_CFGS[0], LAUNCH_CFGS[2]):
        nc = bacc.Bacc("TRN2", target_bir_lowering=False, debug=False,
                       num_devices=NCORES)
        _build_launch(nc, cfgs)
        nc.compile()
        jfs.append(_bass_jit(nc))

    def _reshard_local(x):
        # x: [C*SLAB, Y*Z] owned planes -> [P, YP*ZP] padded with halos
        v = x.reshape(C, SLAB, Y, Z)
        snd_r = v[:, SLAB - H:]
        snd_l = v[:, 0:H]
        perm_r = [(i, (i + 1) % NCORES) for i in range(NCORES)]
        perm_l = [(i, (i - 1) % NCORES) for i in range(NCORES)]
        rcv_l = jax.lax.ppermute(snd_r, "core", perm_r)
        rcv_r = jax.lax.ppermute(snd_l, "core", perm_l)
        full = jnp.concatenate([rcv_l, v, rcv_r], axis=1)  # [C, 24+2H, Y, Z]
        full = jnp.pad(full, ((0, 0), (0, PX - SLAB - 2 * H),
                              (PAD, PAD), (PAD, PAD)))
        return full.reshape(P, YP * ZP)

    jresh = jax.jit(shard_map(_reshard_local, mesh=mesh,
                              in_specs=(PartitionSpec("core"),),
                              out_specs=PartitionSpec("core")))

    (jfA, _), (jfB, _) = jfs
    zin = jax.device_put(
        np.zeros((NCORES * P, YP * ZP), BF16NP), shard)
    zout = jax.device_put(
        np.zeros((NCORES * C * SLAB, Y * Z), BF16NP), shard)

    def pipe(d_own_np):
        d = jax.device_put(d_own_np, shard)
        for jf in (jfA, jfA, jfB):
            t = jresh(d)
            (d,) = jf(t, zout)
        return np.asarray(jax.block_until_ready(d))

    _RUNNERS["pipe"] = pipe
    return pipe


def kernel(velocity: np.ndarray) -> np.ndarray:
    v = np.asarray(velocity, dtype=np.float32).reshape(X, Y, Z, C)
    d = (np.moveaxis(v, -1, 0) / np.float32(2.0 ** STEPS)).astype(BF16NP)
    pipe = _get_pipeline()
    d_own = np.ascontiguousarray(
        d.reshape(C, NCORES, SLAB, Y * Z).transpose(1, 0, 2, 3)
    ).reshape(NCORES * C * SLAB, Y * Z)
    out = pipe(d_own)
    d = np.ascontiguousarray(
        out.reshape(NCORES, C, SLAB, Y, Z).transpose(1, 0, 2, 3, 4)
    ).reshape(C, X, Y, Z)
    df = np.moveaxis(d.astype(np.float32), 0, -1)  # (X, Y, Z, C)
    gx, gy, gz = np.meshgrid(np.arange(X, dtype=np.float32),
                             np.arange(Y, dtype=np.float32),
                             np.arange(Z, dtype=np.float32), indexing="ij")
    df[..., 0] += gx
    df[..., 1] += gy
    df[..., 2] += gz
    return df.reshape(1, X, Y, Z, C)
